# revision 68
# baseline (speedup 1.0000x reference)
"""MinibatchDiscrimination Trainium2 kernel (8-core SPMD), v2.

Computes: M = einsum('nf,fbi->nbi', x, T); l1[n,j,b] = sum_i |M[n,b,i]-M[j,b,i]|;
out = concat([x, sum_j exp(-l1) - 1], axis=1).

Sharding (symmetric pairs): every core runs the SAME NEFF; core c receives
x row-rotated by -32*c. Local row n' (global n = 32c+n') pairs only with the
window j' in [n'+1, n'+128] (ring distance 1..128), so each unordered pair is
computed exactly once fleet-wide, except distance-128 pairs (twice; the dup
output corrects that). Row sums come from accum_out; the mirror (column)
contributions are accumulated on-device and merged by the host gather (the
gather itself is the "all-reduce" -- no collectives).

Device algorithm (per core), heavy data fp16 (|d| = 2*relu(d) - d since the
DVE TensorScalar ALU has no abs op):
 1. PE: Mt chunks [128=(8b,16i), 256 j] = (T.reshape(512,4096) chunk)^T @ x^T,
    fp32 PSUM -> fp16 tiles mt[c] (separate tiles keep the DVE 4x fast mode);
    fp32 cols bank = rounded n'-columns. SM = Wsum^T @ x^T (host precomputes
    Wsum = T.sum(axis=2)) -> smneg = -SM fp16.
 2. Per (n', c): DVE (3 of 4) / GPSIMD (1 of 4) tensor_scalar(sub, max 0)
    -> A = relu(mt_c[:, window] - col), batched 4 chunks per tile.
 3. PE: 2*block-ones E_(c%16) matmuls accumulate 2R over i into psum lo/hi
    [128 b, 128 window] fp32, plus one identity matmul adding -SM
    -> psum = 2R - SM = l1 - SM[b,n'].
 4. ACT: activation(Exp, scale=-1, bias=-SM[:,n'], accum_out) -> fp16 exp
    tile + row-sum column. PE identity-matmuls accumulate the exp tiles into
    persistent PSUM banks (column partials); ACT saves the distance-128
    dup column. Host: out[n] = row - dup + rolled column partials.

Hardcoded shapes: x (256, 512) f32, T (512, 256, 16) f32.
"""
import sys

sys.path.insert(0, "/opt/trn_rl_repo")

import numpy as np

N = 256       # batch
F = 512       # in features
B = 256       # discrimination features
I = 16        # intermediate features
NCORES = 8
NPER = N // NCORES   # 32 rows per core
KCH = F // 128       # 4 contraction chunks
CCH = (B * I) // 128  # 32 (b,i)-partition chunks
NE = 128 // 8         # 16 distinct E matrices

_compiled = None


def _build():
    import concourse.bacc as bacc
    import concourse.tile as tile
    from concourse import mybir

    F32 = mybir.dt.float32
    BF = mybir.dt.float16
    nc = bacc.Bacc(trn_type="TRN2", target_bir_lowering=False)

    xT_d = nc.dram_tensor("xT", [F, N], BF, kind="ExternalInput")
    wn_d = nc.dram_tensor("Wn", [F, B * I], BF, kind="ExternalInput")
    ws_d = nc.dram_tensor("Wsum", [F, B], BF, kind="ExternalInput")
    # E slots 0..NE-1: 2*block-ones (reduce-over-i, x2 of |d| = 2*relu(d) - d),
    # slot NE: identity (adds -SM into the psum accumulation).
    e_d = nc.dram_tensor("E", [NE + 1, 128, 128], BF, kind="ExternalInput")
    out_d = nc.dram_tensor("out_row", [2, 128, NPER], F32, kind="ExternalOutput")
    dup_d = nc.dram_tensor("out_dup", [2, 128, NPER], F32, kind="ExternalOutput")
    col_d = nc.dram_tensor("out_col", [2, 128, N], F32, kind="ExternalOutput")

    WN_SPLIT = 8
    with tile.TileContext(nc) as tc:
        with (
            tc.tile_pool(name="wpool", bufs=1) as wpool,
            tc.tile_pool(name="apool", bufs=24) as apool,
            tc.tile_pool(name="epool", bufs=3) as epool,
            tc.tile_pool(name="opool", bufs=1) as opool,
            tc.tile_pool(name="psmt", bufs=6, space="PSUM") as psmt,
            tc.tile_pool(name="psacc", bufs=3, space="PSUM") as psacc,
        ):
            xt_all = wpool.tile([128, KCH, N], BF, name="xt_all")
            nc.sync.dma_start(xt_all[:], xT_d[:].rearrange("(k p) n -> p k n", k=KCH))
            ws_all = wpool.tile([128, KCH, B], BF, name="ws_all")
            nc.scalar.dma_start(ws_all[:], ws_d[:].rearrange("(k p) b -> p k b", k=KCH))
            # Wn split into ramped column groups across both HWDGE queues:
            # small first groups let phase-1 matmuls start earliest.
            e_all = wpool.tile([128, NE + 1, 128], BF, name="e_all")
            nc.scalar.dma_start(e_all[:], e_d[:].rearrange("e p q -> p e q"))
            wn_all = wpool.tile([128, KCH, B * I], BF, name="wn_all")
            dma_engines = [nc.sync, nc.scalar]
            bounds = [0, 128, 256, 512, 1024, 1536, 2048, 3072, 4096]
            for g in range(len(bounds) - 1):
                lo, hi = bounds[g], bounds[g + 1]
                dma_engines[g % len(dma_engines)].dma_start(
                    wn_all[:, :, lo:hi],
                    wn_d[:, lo:hi].rearrange("(k p) c -> p k c", k=KCH),
                )

            # Mt chunks as separate tiles: tensor_scalar 4x mode requires
            # operand base offset 0, so each chunk gets its own allocation.
            JW = NPER + 128  # only j' < 160 is ever read in phase 2
            mt = [
                wpool.tile([128, JW], BF, name=f"mt{c}", tag=f"mt{c}")
                for c in range(CCH)
            ]
            cols = wpool.tile([128, CCH, NPER], F32, name="cols")

            # SM[b, j] = sum_i M[j, b, i] = Wsum^T @ x^T (host precomputes
            # Wsum = T.sum(axis=2)); smneg = -SM as fp16.
            smneg = wpool.tile([128, 2, NPER + 128], BF, name="smneg")
            smneg_cols = wpool.tile([128, 2, NPER], F32, name="smneg_cols")
            for half in range(2):
                ps_sm = psmt.tile([128, N], F32, name="ps_sm", tag="pt")
                for k in range(KCH):
                    nc.tensor.matmul(
                        ps_sm[:, 0:JW],
                        ws_all[:, k, 128 * half : 128 * (half + 1)],
                        xt_all[:, k, 0:JW],
                        start=(k == 0),
                        stop=(k == KCH - 1),
                    )
                nc.scalar.mul(out=smneg[:, half, :], in_=ps_sm[:, 0:JW], mul=-1.0)
                nc.scalar.copy(out=smneg_cols[:, half, :], in_=smneg[:, half, 0:NPER])

            # Phase 1: Mt = Wn^T @ x^T in interleaved (8b,16i)-partition layout
            for c in range(CCH):
                pt = psmt.tile([128, N], F32, name="pt", tag="pt")
                for k in range(KCH):
                    nc.tensor.matmul(
                        pt[:, 0:JW],
                        wn_all[:, k, 128 * c : 128 * (c + 1)],
                        xt_all[:, k, 0:JW],
                        start=(k == 0),
                        stop=(k == KCH - 1),
                    )
                # mt copies on ACT (slack engine); cols on DVE so the
                # steady-state DVE ts stream carries no copy load.
                nc.scalar.copy(out=mt[c][:], in_=pt[:, 0:JW])
                # cols = ROUNDED values (kept for scalar consistency)
                nc.vector.tensor_copy(out=cols[:, c, :], in_=mt[c][:, 0:NPER])

            out_row = [
                opool.tile([128, NPER], F32, name=f"out_row{h}") for h in range(2)
            ]
            out_dup = [
                opool.tile([128, NPER], F32, name=f"out_dup{h}") for h in range(2)
            ]
            # column-partial accumulators live in PSUM; PE identity-matmuls
            # accumulate the fp16 exp tiles (all-PE accumulation chains are
            # HW-safe). Bank zeroed by one full-width matmul of zeros.
            zero_pad = wpool.tile([128, N], BF, name="zero_pad")
            nc.vector.memset(zero_pad[:], 0.0)
            col_ps = [
                psacc.tile([128, N], F32, name=f"col_ps{h}", tag=f"colps{h}", bufs=1)
                for h in range(2)
            ]
            for h in range(2):
                nc.tensor.matmul(
                    col_ps[h][:], e_all[:, NE, :], zero_pad[:],
                    start=True, stop=False, skip_group_check=True,
                )

            # Phase 2 (symmetric pairs): row n' only pairs with the window
            # j' in [n'+1, n'+128] (distance 1..128 on the rotated ring), so
            # every unordered pair is computed once fleet-wide except
            # distance-128 pairs (twice; the dup output corrects on host).
            # l1[b,jj] = 2*sum_i relu(d_i) - sum_i d_i over the window;
            # exp via Exp(psum * -1 + bias). Row sums via accum_out; mirror
            # (column) contributions accumulate into col_acc; dup = the
            # distance-128 column of the exp tile.
            W = 128
            for np_ in range(NPER):
                off = np_ + 1
                ps_halves = [
                    psmt.tile([128, W], F32, name="ps_lo", tag="pt"),
                    psmt.tile([128, W], F32, name="ps_hi", tag="pt"),
                ]
                for c in range(CCH):
                    if c % 4 == 0:
                        a_grp = apool.tile([128, 4, W], BF, name="a_grp", tag="a", bufs=12)
                    a = a_grp[:, c % 4, :]
                    ts_eng = nc.gpsimd if c % 4 == 3 else nc.vector
                    ts_eng.tensor_scalar(
                        out=a,
                        in0=mt[c][:, off : off + W],
                        scalar1=cols[:, c, np_ : np_ + 1],
                        scalar2=0.0,
                        op0=mybir.AluOpType.subtract,
                        op1=mybir.AluOpType.max,
                    )
                    nc.tensor.matmul(
                        ps_halves[c // NE][:],
                        e_all[:, c % NE, :],
                        a,
                        start=(c % NE == 0),
                        stop=False,
                    )
                for half in range(2):
                    ps = ps_halves[half]
                    nc.tensor.matmul(
                        ps[:],
                        e_all[:, NE, :],
                        smneg[:, half, off : off + W],
                        start=False,
                        stop=True,
                    )
                    esc = epool.tile([128, W], BF, name="esc", tag="esc")
                    nc.scalar.activation(
                        out=esc[:],
                        in_=ps[:],
                        func=mybir.ActivationFunctionType.Exp,
                        scale=-1.0,
                        bias=smneg_cols[:, half, np_ : np_ + 1],
                        accum_out=out_row[half][:, np_ : np_ + 1],
                    )
                    # mirror contributions for rows j' = off..off+W via PE
                    nc.tensor.matmul(
                        col_ps[half][:, off : off + W],
                        e_all[:, NE, :],
                        esc[:],
                        start=False, stop=(np_ == NPER - 1),
                        skip_group_check=True,
                    )
                    # distance-128 duplicate column
                    nc.scalar.copy(
                        out=out_dup[half][:, np_ : np_ + 1],
                        in_=esc[:, W - 1 : W],
                    )

            for h in range(2):
                col_sb = opool.tile([128, N], F32, name=f"col_sb{h}")
                nc.scalar.copy(out=col_sb[:], in_=col_ps[h][:])
                nc.sync.dma_start(out_d[h], out_row[h][:])
                nc.sync.dma_start(dup_d[h], out_dup[h][:])
                nc.sync.dma_start(col_d[h], col_sb[:])

    nc.finalize()
    return nc


def _get_compiled():
    global _compiled
    if _compiled is None:
        _compiled = _build()
    return _compiled


def _prep_inputs(x, T):
    """Per-core input maps. Core c gets x row-rotated by -NPER*c."""
    bf = np.float16
    wn = np.ascontiguousarray(T.reshape(F, B * I)).astype(bf)
    wsum = T.reshape(F, B, I).sum(axis=2).astype(bf)
    e = np.zeros((NE + 1, 128, 128), dtype=bf)
    for ei in range(NE):
        for p in range(128):
            e[ei, p, 8 * ei + p // 16] = 2.0
    e[NE] = np.eye(128, dtype=bf)
    in_maps = []
    for c in range(NCORES):
        xr = np.roll(x, -NPER * c, axis=0)
        xT = np.ascontiguousarray(xr.T).astype(bf)
        in_maps.append({"xT": xT, "Wn": wn, "Wsum": wsum, "E": e})
    return in_maps


def _assemble(x, results):
    """Combine symmetric-pair partials: every unordered pair was computed by
    exactly one core (distance-128 pairs by both, corrected via dup). The
    reference's (sum_j - 1) equals our self-pair-free sum directly."""
    out_disc = np.zeros((N, B), dtype=np.float32)
    for c, res in enumerate(results):
        rows = res["out_row"].transpose(2, 0, 1).reshape(NPER, B)
        dups = res["out_dup"].transpose(2, 0, 1).reshape(NPER, B)
        out_disc[NPER * c : NPER * (c + 1), :] += rows - dups
        colg = res["out_col"].reshape(B, N)  # [b, local j']
        # local j' -> global row (j' + NPER*c) % N
        out_disc += np.roll(colg.T, NPER * c, axis=0)
    return np.concatenate([x.astype(np.float32), out_disc], axis=1)


def kernel_run(x, T, trace=False):
    from concourse.bass_utils import run_bass_kernel_spmd

    nc = _get_compiled()
    in_maps = _prep_inputs(x, T)
    res = run_bass_kernel_spmd(nc, in_maps, core_ids=list(range(NCORES)), trace=trace)
    return _assemble(x, res.results), res


def kernel(x, T):
    out, _ = kernel_run(x, T, trace=False)
    return out
